# revision 31
# baseline (speedup 1.0000x reference)
"""Multi-head causal attention with RoPE on 8 Trainium2 NeuronCores.

Sharding: core = batch(2) x head-group(4).  Each core computes the q/k/v
projections for its 4 heads (256 of 1024 channels), RoPE, causal attention,
and a partial o_proj against its 256 rows of Wo^T; the host sums the 4
partials per batch element.

Everything is bf16 on the wire and in SBUF (psum accumulation stays f32):
halves HBM traffic vs f32r and enables PE fast-weight-load.

Device layouts (per core):
  xT       [1024, 2048] bf16   x[b].T
  wqT/wkT/wvT [128, 8*256] bf16  K-block-major W.T slices (wq pre-scaled 1/8)
  woT      [128, 2*1024] bf16  c-block-major Wo[:, g].T
  cosT2/sinT2 [128, 2048] bf16 rope tables, stacked twice (head pair rows)
  rotT     [128, 128]  bf16    blockdiag(R,R).T, R = rotate_half matrix
  tri01/ident [128, 128] bf16  lower-incl-diag 0/1 mask; identity
  out      [2048, 1024] bf16   partial (x @ Wo_g partial), host-summed

Attention: score pairs run as concurrent 64x128 row tiles of the PE array
(tile_position (0,0)/(64,0) -- heads 2p/2p+1 live in partitions 0:64/64:128).
The J=0 score tiles are interleaved one-per-psv-chain into the v-projection
phase so the scalar engine (exp -- the attention bottleneck) is already
saturated while the PE streams dense projection matmuls.  The causal diag
block is masked post-exp by a 0/1 triangle multiply on gpsimd.  attn@v runs
in NATURAL orientation: per q-block jg, out[q 128, 65] accumulates
matmul(lhsT=et[i][:, q-block], rhs=v[:, i, h]) over k-blocks i<=jg -- exact
causal trim, ones column of v carries the softmax denominator, normalize
straight from psum.  o_proj runs per 1024-seq super; super 0 overlaps the
super-1 attention.
"""
import os
import sys

sys.path.insert(0, "/opt/trn_rl_repo")

import numpy as np
import ml_dtypes

import concourse.bacc as bacc
import concourse.mybir as mybir
from concourse import tile
from concourse.bass_utils import run_bass_kernel_spmd

F32 = mybir.dt.float32
BF16 = mybir.dt.bfloat16

D_MODEL = 1024
N_HEADS = 16
HEAD_DIM = 64
SEQ = 2048
BATCH = 2
ROPE_THETA = 10000.0

NB = SEQ // 128          # 16 s-blocks of 128
NSUP = SEQ // 1024       # 2 s-supers of 1024
HPG = 4                  # heads per group (per core)
CPG = HPG * HEAD_DIM     # 256 channels per group

_CACHE = {}
LAST_RESULT = None       # test harness reads exec_time_ns from here


def _build_nc(causal: bool):
    nc = bacc.Bacc("TRN2", target_bir_lowering=False, debug=False, num_devices=8)

    xT_d = nc.declare_dram_parameter("xT", [D_MODEL, SEQ], BF16, isOutput=False)
    wq_d = nc.declare_dram_parameter("wqT", [128, 8 * CPG], BF16, isOutput=False)
    wk_d = nc.declare_dram_parameter("wkT", [128, 8 * CPG], BF16, isOutput=False)
    wv_d = nc.declare_dram_parameter("wvT", [128, 8 * CPG], BF16, isOutput=False)
    wo_d = nc.declare_dram_parameter("woT", [128, 2 * D_MODEL], BF16, isOutput=False)
    cos_d = nc.declare_dram_parameter("cosT2", [128, SEQ], BF16, isOutput=False)
    sin_d = nc.declare_dram_parameter("sinT2", [128, SEQ], BF16, isOutput=False)
    rot_d = nc.declare_dram_parameter("rotT", [128, 128], BF16, isOutput=False)
    tri_d = nc.declare_dram_parameter("tri01", [128, 128], BF16, isOutput=False)
    id_d = nc.declare_dram_parameter("ident", [128, 128], BF16, isOutput=False)
    out_d = nc.declare_dram_parameter("out", [D_MODEL, SEQ], BF16, isOutput=True)

    xT_r = xT_d.rearrange("(kb p) s -> p kb s", p=128)

    with tile.TileContext(nc) as tc:
        with tc.tile_pool(name="res", bufs=1) as res:
            # ---- resident constants ----
            wq_sb = res.tile([128, 8 * CPG], BF16)
            wk_sb = res.tile([128, 8 * CPG], BF16)
            wv_sb = res.tile([128, 8 * CPG], BF16)
            wo_sb = res.tile([128, 2 * D_MODEL], BF16)
            cos_sb = res.tile([128, SEQ], BF16)
            sin_sb = res.tile([128, SEQ], BF16)
            rot_sb = res.tile([128, 128], BF16)
            tri_sb = res.tile([128, 128], BF16)
            id_sb = res.tile([128, 128], BF16)

            # ---- resident activations ----
            qf = res.tile([128, 2 * SEQ], BF16)          # [pair rows, pr*SEQ + s]
            kf = res.tile([128, 2 * SEQ], BF16)
            v_sb = res.tile([128, NB, HPG * 65], BF16)   # per s-block, head-slot 65 cols
            attn = res.tile([128, NB, CPG], BF16)        # attn out, natural [s, c]
            attnT = res.tile([128, 2 * SEQ], BF16)       # attn out transposed [c, cb*SEQ + s]
            nc.vector.memset(v_sb[:, :, 64 : HPG * 65 : 65], 1.0)

            # prewarm the ACT exp table during the DMA/proj phase
            warm = res.tile([128, 1], F32)
            warm2 = res.tile([128, 1], BF16)
            nc.vector.memset(warm[:], 0.0)
            nc.scalar.activation(warm2[:], warm[:], mybir.ActivationFunctionType.Exp)

            EXP = mybir.ActivationFunctionType.Exp

            # att (SBUF) opens before proj so the J0 score/exp tiles can be
            # produced during the projection phase
            with tc.tile_pool(name="att", bufs=1) as att:

                def score_i(p, J, i, pspool, ets):
                    # one k-block of scores for head pair (2p, 2p+1):
                    # two concurrent 64x128 row-tile matmuls + exp per head
                    t = i - 8 * J
                    col0 = max(t, 0) * 128 if causal else 0
                    pair = []
                    for hh in range(2):
                        h = 2 * p + hh
                        et = att.tile(
                            [128, 1024], BF16, name=f"et{h}_{J}_{i}", tag="et", bufs=40
                        )
                        pair.append(et)
                        ets[hh].append(et)
                    psss = ([], [])
                    for nh in range(2):
                        lo = max(col0, nh * 512)
                        hi = (nh + 1) * 512
                        if lo >= hi:
                            continue
                        for hh in range(2):
                            off = hh * 64
                            qT_h = qf[off : off + 64, p * SEQ : (p + 1) * SEQ]
                            kT_h = kf[off : off + 64, p * SEQ : (p + 1) * SEQ]
                            pss = pspool.tile([128, 512], F32, name="pss", tag="pb")
                            nc.tensor.matmul(
                                pss[:, 0 : hi - lo],
                                kT_h[:, i * 128 : (i + 1) * 128],
                                qT_h[:, J * 1024 + lo : J * 1024 + hi],
                                start=True,
                                stop=True,
                                tile_position=(off, 0),
                            )
                            psss[hh].append((pss, lo, hi))
                    for hh in range(2):
                        for pss, lo, hi in psss[hh]:
                            nc.scalar.activation(pair[hh][:, lo:hi], pss[:, 0 : hi - lo], EXP)
                        if causal and 0 <= t <= 7:
                            # zero the above-diagonal wedge of the diag block
                            nc.gpsimd.tensor_mul(
                                pair[hh][:, t * 128 : (t + 1) * 128],
                                pair[hh][:, t * 128 : (t + 1) * 128],
                                tri_sb[:],
                            )

                # ================= projections + rope (+ J0 scores) =========
                with (
                    tc.tile_pool(name="proj", bufs=2) as proj,
                    tc.tile_pool(name="psP", bufs=8, space="PSUM") as psP,
                ):
                    xts = {}
                    for sup in range(NSUP):
                        s0 = sup * 1024
                        xts[sup] = []
                        for kb in range(8):
                            xt = proj.tile(
                                [128, 1024], BF16, name=f"xt{sup}_{kb}", tag="xt", bufs=17
                            )
                            nc.sync.dma_start(xt[:], xT_r[:, kb, s0 : s0 + 1024])
                            xts[sup].append(xt)
                            if sup == 0:
                                # PE-critical streams first; DVE-only tables and
                                # late-phase constants trail the x tiles
                                if kb == 0:
                                    nc.sync.dma_start(wq_sb[:], wq_d[:])
                                elif kb == 1:
                                    nc.sync.dma_start(wk_sb[:], wk_d[:])
                                elif kb == 2:
                                    nc.sync.dma_start(rot_sb[:], rot_d[:])
                                elif kb == 3:
                                    nc.sync.dma_start(cos_sb[:], cos_d[:])
                                elif kb == 4:
                                    nc.sync.dma_start(sin_sb[:], sin_d[:])
                                elif kb == 5:
                                    nc.sync.dma_start(wv_sb[:], wv_d[:])
                                elif kb == 6:
                                    nc.sync.dma_start(tri_sb[:], tri_d[:])
                                    nc.sync.dma_start(id_sb[:], id_d[:])
                                elif kb == 7:
                                    nc.sync.dma_start(wo_sb[:], wo_d[:])

                    # q/k chains + rope for both supers
                    for sup in range(NSUP):
                        s0 = sup * 1024
                        xp = xts[sup]
                        for tens, (w_sb, outf) in enumerate(((wq_sb, qf), (wk_sb, kf))):
                            qraws = []
                            for pr in range(2):
                                qraw = proj.tile(
                                    [128, 1024], BF16, name="qraw", tag="qraw", bufs=3
                                )
                                for nh in range(2):
                                    psq = psP.tile([128, 512], F32, name="psq", tag="pb")
                                    for kb in range(8):
                                        lhs = w_sb[
                                            :, kb * CPG + pr * 128 : kb * CPG + (pr + 1) * 128
                                        ]
                                        nc.tensor.matmul(
                                            psq[:],
                                            lhs,
                                            xp[kb][:, nh * 512 : (nh + 1) * 512],
                                            start=(kb == 0),
                                            stop=(kb == 7),
                                        )
                                    nc.vector.tensor_copy(
                                        qraw[:, nh * 512 : (nh + 1) * 512], psq[:]
                                    )
                                qraws.append(qraw)
                            for pr in range(2):
                                qraw = qraws[pr]
                                for nh in range(2):
                                    psr = psP.tile([128, 512], F32, name="psr", tag="pb")
                                    nc.tensor.matmul(
                                        psr[:],
                                        rot_sb[:],
                                        qraw[:, nh * 512 : (nh + 1) * 512],
                                        start=True,
                                        stop=True,
                                    )
                                    c0 = s0 + nh * 512
                                    t1 = proj.tile([128, 512], BF16, name="t1", tag="t1", bufs=3)
                                    nc.vector.tensor_mul(
                                        t1[:],
                                        qraw[:, nh * 512 : (nh + 1) * 512],
                                        cos_sb[:, c0 : c0 + 512],
                                    )
                                    t2 = proj.tile([128, 512], F32, name="t2", tag="t2", bufs=3)
                                    nc.vector.tensor_mul(t2[:], psr[:], sin_sb[:, c0 : c0 + 512])
                                    dst = outf[:, pr * SEQ + c0 : pr * SEQ + c0 + 512]
                                    nc.vector.tensor_add(dst, t1[:], t2[:])

                    # v projections, interleaved with the J0 score tiles: the
                    # psv chains keep the PE dense while ACT chews the exps
                    e00, e10 = ([], []), ([], [])
                    n_i0 = 8 if causal else NB
                    sjobs = [(0, 0, i, e00) for i in range(n_i0)] + [
                        (1, 0, i, e10) for i in range(n_i0)
                    ]
                    vjobs = [(sup, sbi) for sup in range(NSUP) for sbi in range(8)]
                    sk = 0
                    for vk, (sup, sbi) in enumerate(vjobs):
                        xp = xts[sup]
                        blk = sup * 8 + sbi
                        psv = psP.tile([128, CPG], F32, name="psv", tag="pb")
                        for kb in range(8):
                            nc.tensor.matmul(
                                psv[:],
                                xp[kb][:, sbi * 128 : (sbi + 1) * 128],
                                wv_sb[:, kb * CPG : (kb + 1) * CPG],
                                start=(kb == 0),
                                stop=(kb == 7),
                            )
                        nc.vector.tensor_copy(
                            v_sb[:, blk, :].rearrange("p (h c) -> p h c", h=HPG)[:, :, 0:64],
                            psv[:].rearrange("p (h c) -> p h c", h=HPG),
                        )
                        # distribute the score jobs evenly across the v chains
                        want = (vk + 1) * len(sjobs) // len(vjobs)
                        while sk < want:
                            p, J, i, ets = sjobs[sk]
                            score_i(p, J, i, psP, ets)
                            sk += 1

                # ================= attention =================
                with tc.tile_pool(name="psA", bufs=6, space="PSUM") as psA:

                    def emit_scores_pair(p, J):
                        ets = ([], [])
                        n_i = 8 * J + 8 if causal else NB
                        for i in range(n_i):
                            score_i(p, J, i, psA, ets)
                        return ets

                    def emit_chains(h, J, ets):
                        for jp in range(8):
                            jg = 8 * J + jp
                            n_i = jg + 1 if causal else NB
                            pav = psA.tile([128, 65], F32, name="pav", tag="pav", bufs=2)
                            for i in range(n_i):
                                nc.tensor.matmul(
                                    pav[:],
                                    ets[i][:, jp * 128 : (jp + 1) * 128],
                                    v_sb[:, i, h * 65 : h * 65 + 65],
                                    start=(i == 0),
                                    stop=(i == n_i - 1),
                                )
                            rec = att.tile([128, 1], F32, name="rec", tag="rec", bufs=4)
                            nc.vector.reciprocal(rec[:], pav[:, 64:65])
                            nc.vector.tensor_scalar_mul(
                                attn[:, jg, h * 64 : (h + 1) * 64], pav[:, 0:64], rec[:]
                            )

                    def emit_oproj(J):
                        for jp in range(8):
                            jg = 8 * J + jp
                            for cb in range(2):
                                ptt = psA.tile([128, 128], BF16, name="ptt", tag="pb")
                                nc.tensor.transpose(
                                    ptt[:], attn[:, jg, cb * 128 : (cb + 1) * 128], id_sb[:]
                                )
                                nc.vector.tensor_copy(
                                    attnT[:, cb * SEQ + jg * 128 : cb * SEQ + (jg + 1) * 128],
                                    ptt[:],
                                )
                        for db in range(8):
                            psos = [
                                psA.tile([128, 512], F32, name=f"pso{db}_{ss}", tag="pb")
                                for ss in range(2)
                            ]
                            for cb in range(2):
                                lhs = wo_sb[
                                    :, cb * D_MODEL + db * 128 : cb * D_MODEL + (db + 1) * 128
                                ]
                                for ss in range(2):
                                    nc.tensor.matmul(
                                        psos[ss][:],
                                        lhs,
                                        attnT[
                                            :,
                                            cb * SEQ + J * 1024 + ss * 512 : cb * SEQ
                                            + J * 1024
                                            + (ss + 1) * 512,
                                        ],
                                        start=(cb == 0),
                                        stop=(cb == 1),
                                    )
                            osb = att.tile([128, 1024], BF16, name="osb", tag="osb", bufs=2)
                            nc.vector.tensor_copy(osb[:, 0:512], psos[0][:])
                            nc.scalar.copy(osb[:, 512:1024], psos[1][:])
                            nc.sync.dma_start(
                                out_d[db * 128 : (db + 1) * 128, J * 1024 : (J + 1) * 1024],
                                osb[:],
                            )

                    # J0 exps were produced during the projection phase; the
                    # J1 score pair-batches lead their chain consumers.  et
                    # bufs=40 slot-reuse stays ahead of emission order
                    # (verified: slot k+40's consumer precedes allocation
                    # k+40 in PE order).
                    emit_chains(0, 0, e00[0])
                    emit_chains(1, 0, e00[1])
                    emit_chains(2, 0, e10[0])
                    emit_chains(3, 0, e10[1])
                    e01h = emit_scores_pair(0, 1)
                    emit_oproj(0)
                    emit_chains(0, 1, e01h[0])
                    emit_chains(1, 1, e01h[1])
                    e21h = emit_scores_pair(1, 1)
                    emit_chains(2, 1, e21h[0])
                    emit_chains(3, 1, e21h[1])
                    emit_oproj(1)

    nc.compile()
    return nc


def _host_tables():
    inv_freq = 1.0 / (ROPE_THETA ** (np.arange(0, HEAD_DIM, 2, dtype=np.float64) / HEAD_DIM))
    ang = np.arange(SEQ, dtype=np.float64)[:, None] * inv_freq[None, :]  # [S, 32]
    cos_h = np.cos(ang)
    sin_h = np.sin(ang)
    cos_full = np.concatenate([cos_h, cos_h], axis=1).astype(np.float32)  # [S, 64]
    sin_full = np.concatenate([sin_h, sin_h], axis=1).astype(np.float32)
    cosT2 = np.ascontiguousarray(np.vstack([cos_full.T, cos_full.T]))  # [128, S]
    sinT2 = np.ascontiguousarray(np.vstack([sin_full.T, sin_full.T]))
    # rotate_half matrix R [64,64]: (Rq)[j] = -q[j+32] (j<32), q[j-32] (j>=32)
    R = np.zeros((64, 64), np.float32)
    for jj in range(32):
        R[jj, jj + 32] = -1.0
        R[jj + 32, jj] = 1.0
    Rp = np.zeros((128, 128), np.float32)
    Rp[0:64, 0:64] = R
    Rp[64:128, 64:128] = R
    rotT = np.ascontiguousarray(Rp.T)
    return cosT2, sinT2, rotT


def _kb_major(wT):
    # [1024, C] -> [128, 8*C] with K-block-major columns
    C = wT.shape[1]
    return np.ascontiguousarray(wT.reshape(8, 128, C).transpose(1, 0, 2).reshape(128, 8 * C))


def _np_reference(x, mask, Wq, Wk, Wv, Wo):
    B, S, D = x.shape
    cosT2, sinT2, _ = _host_tables()
    cos = cosT2[:64].T[None, :, None, :]  # [1,S,1,64]
    sin = sinT2[:64].T[None, :, None, :]
    q = (x @ Wq.T).reshape(B, S, N_HEADS, HEAD_DIM)
    k = (x @ Wk.T).reshape(B, S, N_HEADS, HEAD_DIM)
    v = (x @ Wv.T).reshape(B, S, N_HEADS, HEAD_DIM)

    def rot(t):
        return np.concatenate([-t[..., 32:], t[..., :32]], axis=-1)

    q = q * cos + rot(q) * sin
    k = k * cos + rot(k) * sin
    sc = np.einsum("bqhd,bkhd->bhqk", q, k) / np.sqrt(HEAD_DIM)
    sc = np.where(mask[None, None], -np.inf, sc)
    sc = sc - sc.max(-1, keepdims=True)
    e = np.exp(sc)
    a = e / e.sum(-1, keepdims=True)
    o = np.einsum("bhqk,bkhd->bqhd", a, v).reshape(B, S, D)
    return (o @ Wo.T).astype(np.float32)


def _bf16(a):
    return np.ascontiguousarray(a).astype(ml_dtypes.bfloat16)


def kernel(x, mask, Wq, Wk, Wv, Wo):
    global LAST_RESULT
    x = np.asarray(x, np.float32)
    mask = np.asarray(mask, bool)
    Wq = np.asarray(Wq, np.float32)
    Wk = np.asarray(Wk, np.float32)
    Wv = np.asarray(Wv, np.float32)
    Wo = np.asarray(Wo, np.float32)

    causal_mask = np.triu(np.ones((SEQ, SEQ), bool), 1)
    if np.array_equal(mask, causal_mask):
        causal = True
    else:
        return _np_reference(x, mask, Wq, Wk, Wv, Wo)

    if causal not in _CACHE:
        _CACHE[causal] = _build_nc(causal)
    nc = _CACHE[causal]

    cosT2, sinT2, rotT = _host_tables()
    # 0/1 mask for the diagonal 128-block in [k_row, q_col] layout:
    # valid (keep) where q >= k, i.e. col >= row
    tri01 = _bf16(
        np.where(np.arange(128)[None, :] >= np.arange(128)[:, None], 1.0, 0.0).astype(np.float32)
    )
    ident = _bf16(np.eye(128, dtype=np.float32))

    in_maps = []
    for b in range(BATCH):
        xT = _bf16(x[b].T)
        for g in range(4):
            sl = slice(g * CPG, (g + 1) * CPG)
            in_maps.append(
                {
                    "xT": xT,
                    "wqT": _bf16(_kb_major(np.ascontiguousarray((Wq[sl] / np.sqrt(HEAD_DIM)).T))),
                    "wkT": _bf16(_kb_major(np.ascontiguousarray(Wk[sl].T))),
                    "wvT": _bf16(_kb_major(np.ascontiguousarray(Wv[sl].T))),
                    "woT": _bf16(
                        Wo[:, sl].T.reshape(2, 128, D_MODEL).transpose(1, 0, 2).reshape(128, 2 * D_MODEL)
                    ),
                    "cosT2": _bf16(cosT2),
                    "sinT2": _bf16(sinT2),
                    "rotT": _bf16(rotT),
                    "tri01": tri01,
                    "ident": ident,
                }
            )

    trace = os.environ.get("KERNEL_TRACE", "0") == "1"
    res = run_bass_kernel_spmd(nc, in_maps, list(range(8)), trace=trace)
    LAST_RESULT = res

    out = np.zeros((BATCH, SEQ, D_MODEL), np.float32)
    for b in range(BATCH):
        for g in range(4):
            out[b] += res.results[b * 4 + g]["out"].astype(np.float32).T
    return out
